# revision 4
# baseline (speedup 1.0000x reference)
"""Bahdanau attention on 8 Trainium2 NeuronCores.

Full inputs in, full outputs out. Batch (B=32) is sharded 4-per-core
(data parallel); all weights are replicated. Per core the kernel computes,
for each of its 4 batches:

    e_projT[k, s] = sum_h U_w[k, h] * enc[b, s, h]          (bf16 PE matmul)
    t[k, s]       = tanh(e_projT[k, s] + h_proj[b, k] + W_b[k] + U_b[k])
    scores[s]     = sum_k V[k] * t[k, s]                    (fp32 PE matmul)
    a[s]          = exp(scores[s] + V_b)                    (no max needed:
                      |scores| <= ||V||_1 + |V_b| ~ 26, exp fits fp32 easily)
    attention     = a / sum(a)
    context[h]    = sum_s a[s] * enc[b, s, h] / sum(a)      (bf16 PE matmul)

The h-contraction needs encoder tiles with h on partitions, while the
s-contraction needs s on partitions, so the host passes the encoder twice
(bf16 transposed + bf16 natural); 32 MB/core of DMA vs ~300 us of PE work.
"""

from contextlib import ExitStack

import numpy as np
import ml_dtypes

import concourse.bass as bass  # noqa: F401  (engine types resolve through bacc)
import concourse.mybir as mybir
from concourse import tile, bacc
from concourse.bass_utils import run_bass_kernel_spmd

BF16 = mybir.dt.bfloat16
F32 = mybir.dt.float32
AF = mybir.ActivationFunctionType

B, S, H = 32, 2048, 1024
NCORES = 8
BL = B // NCORES      # 4 batches per core
P = 128
KC = H // P           # 8 contraction chunks
MO = H // P           # 8 output-row chunks
SBLK = 4              # s-blocks per batch
SW = S // SBLK        # 512 columns per s-block
CH = SW // P          # 4 s-chunks of 128 per s-block
NQ = S // P           # 16 s-chunks per batch


def build_program(v_b: float) -> bacc.Bacc:
    nc = bacc.Bacc("TRN2", target_bir_lowering=False, debug=False, num_devices=NCORES)

    encT_d = nc.dram_tensor("encT", [BL, H, S], BF16, kind="ExternalInput")
    encN_d = nc.dram_tensor("encN", [BL, S, H], BF16, kind="ExternalInput")
    uwT_d = nc.dram_tensor("uwT", [H, H], BF16, kind="ExternalInput")
    wwT_d = nc.dram_tensor("wwT", [H, H], F32, kind="ExternalInput")
    hidT_d = nc.dram_tensor("hidT", [H, BL], F32, kind="ExternalInput")
    wub_d = nc.dram_tensor("wub", [P, MO], F32, kind="ExternalInput")
    vcol_d = nc.dram_tensor("vcol", [P, MO], F32, kind="ExternalInput")
    ctx_d = nc.dram_tensor("ctx_out", [BL, H], F32, kind="ExternalOutput")
    attn_d = nc.dram_tensor("attn_out", [BL, S], F32, kind="ExternalOutput")

    with tile.TileContext(nc) as tc, ExitStack() as stack:
        const = stack.enter_context(tc.tile_pool(name="const", bufs=1))

        uw_t = const.tile([P, KC, H], BF16, name="uw_t")
        nc.sync.dma_start(
            out=uw_t[:], in_=uwT_d.ap().rearrange("(kc p) n -> p kc n", p=P)
        )
        wub_t = const.tile([P, MO], F32, name="wub_t")
        nc.sync.dma_start(out=wub_t[:], in_=wub_d.ap())
        v_t = const.tile([P, MO], F32, name="v_t")
        nc.sync.dma_start(out=v_t[:], in_=vcol_d.ap())
        hid_t = const.tile([P, KC, BL], F32, name="hid_t")
        nc.sync.dma_start(
            out=hid_t[:], in_=hidT_d.ap().rearrange("(kc p) b -> p kc b", p=P)
        )
        one_t = const.tile([1, 1], F32, name="one_t")
        nc.vector.memset(one_t[:], 1.0)
        vb_t = const.tile([1, 1], F32, name="vb_t")
        nc.vector.memset(vb_t[:], v_b)
        hb_t = const.tile([P, MO, BL], F32, name="hb_t")

        # ---- phase 0: h_proj[b, :] plus the combined bias, in column layout
        with (
            tc.tile_pool(name="wpool", bufs=1) as wpool,
            tc.tile_pool(name="hp_psum", bufs=2, space="PSUM") as hp_psum,
        ):
            ww_t = wpool.tile([P, KC, H], F32, name="ww_t")
            nc.sync.dma_start(
                out=ww_t[:], in_=wwT_d.ap().rearrange("(kc p) n -> p kc n", p=P)
            )
            for mo in range(MO):
                hp_ps = hp_psum.tile([P, BL], F32, name="hp_ps")
                for kc in range(KC):
                    nc.tensor.matmul(
                        hp_ps[:],
                        ww_t[:, kc, mo * P : (mo + 1) * P],
                        hid_t[:, kc, :],
                        start=(kc == 0),
                        stop=(kc == KC - 1),
                    )
                # hb[:, mo, b] = h_proj[b, mo*128+p] + (W_b + U_b)[mo*128+p]
                nc.scalar.activation(
                    hb_t[:, mo, :], hp_ps[:], AF.Identity, bias=wub_t[:, mo : mo + 1]
                )

        # ---- main pools
        encTp = stack.enter_context(tc.tile_pool(name="encTp", bufs=2))
        encNp = stack.enter_context(tc.tile_pool(name="encNp", bufs=2))
        tanhp = stack.enter_context(tc.tile_pool(name="tanhp", bufs=3))
        rowsp = stack.enter_context(tc.tile_pool(name="rowsp", bufs=2))
        acolp = stack.enter_context(tc.tile_pool(name="acolp", bufs=2))
        outp = stack.enter_context(tc.tile_pool(name="outp", bufs=2))
        ep_psum = stack.enter_context(tc.tile_pool(name="ep_psum", bufs=2, space="PSUM"))
        sc_psum = stack.enter_context(tc.tile_pool(name="sc_psum", bufs=2, space="PSUM"))
        at_psum = stack.enter_context(tc.tile_pool(name="at_psum", bufs=2, space="PSUM"))
        ctx_psum = stack.enter_context(tc.tile_pool(name="ctx_psum", bufs=1, space="PSUM"))

        for b in range(BL):
            rows_t = rowsp.tile([1, S], F32, name="rows_t")
            den4_t = rowsp.tile([1, SBLK], F32, name="den4_t")
            ctx_ps0 = ctx_psum.tile([1, 512], F32, name="ctx_ps0")
            ctx_ps1 = ctx_psum.tile([1, 512], F32, name="ctx_ps1")

            for sb in range(SBLK):
                encT_t = encTp.tile([P, KC, SW], BF16, name="encT_t")
                nc.sync.dma_start(
                    out=encT_t[:],
                    in_=encT_d.ap()[b].rearrange("(kc p) s -> p kc s", p=P)[
                        :, :, sb * SW : (sb + 1) * SW
                    ],
                )
                encN_t = encNp.tile([P, CH, H], BF16, name="encN_t")
                nc.sync.dma_start(
                    out=encN_t[:],
                    in_=encN_d.ap()[b].rearrange("(c p) h -> p c h", p=P)[
                        :, sb * CH : (sb + 1) * CH, :
                    ],
                )

                sc_ps = sc_psum.tile([1, SW], F32, name="sc_ps")
                tanh_tiles = []
                # e_projT + tanh, with the V-dot staggered one mo behind so the
                # PE never stalls waiting on ACT.
                for mo in range(MO):
                    ep_ps = ep_psum.tile([P, SW], F32, name="ep_ps")
                    for kc in range(KC):
                        nc.tensor.matmul(
                            ep_ps[:],
                            uw_t[:, kc, mo * P : (mo + 1) * P],
                            encT_t[:, kc, :],
                            start=(kc == 0),
                            stop=(kc == KC - 1),
                        )
                    tanh_t = tanhp.tile([P, SW], F32, name="tanh_t")
                    nc.scalar.activation(
                        tanh_t[:], ep_ps[:], AF.Tanh, bias=hb_t[:, mo, b : b + 1]
                    )
                    tanh_tiles.append(tanh_t)
                    if mo >= 1:
                        nc.tensor.matmul(
                            sc_ps[:],
                            v_t[:, mo - 1 : mo],
                            tanh_tiles[mo - 1][:],
                            start=(mo - 1 == 0),
                            stop=False,
                            skip_group_check=True,
                        )
                nc.tensor.matmul(
                    sc_ps[:],
                    v_t[:, MO - 1 : MO],
                    tanh_tiles[MO - 1][:],
                    start=False,
                    stop=True,
                    skip_group_check=True,
                )

                # a = exp(scores + V_b); accum_out gives this block's sum(a)
                nc.scalar.activation(
                    rows_t[0:1, sb * SW : (sb + 1) * SW],
                    sc_ps[:],
                    AF.Exp,
                    bias=vb_t[:],
                    accum_out=den4_t[0:1, sb : sb + 1],
                )

                # turn the a-row into 128-deep columns: K=1 matmul against [[1.0]]
                acol_t = acolp.tile([P, CH], BF16, name="acol_t")
                for c in range(CH):
                    q = sb * CH + c
                    at_ps = at_psum.tile([P, 1], F32, name="at_ps")
                    nc.tensor.matmul(
                        at_ps[:],
                        rows_t[0:1, q * P : (q + 1) * P],
                        one_t[:],
                        start=True,
                        stop=True,
                        skip_group_check=True,
                    )
                    nc.vector.tensor_copy(acol_t[:, c : c + 1], at_ps[:])

                # context accumulation over all 16 s-chunks of this batch
                for c in range(CH):
                    q = sb * CH + c
                    st = q == 0
                    sp = q == NQ - 1
                    nc.tensor.matmul(
                        ctx_ps0[:],
                        acol_t[:, c : c + 1],
                        encN_t[:, c, 0:512],
                        start=st,
                        stop=sp,
                        skip_group_check=True,
                    )
                    nc.tensor.matmul(
                        ctx_ps1[:],
                        acol_t[:, c : c + 1],
                        encN_t[:, c, 512:1024],
                        start=st,
                        stop=sp,
                        skip_group_check=True,
                    )

            den_t = rowsp.tile([1, 1], F32, name="den_t")
            nc.vector.reduce_sum(den_t[:], den4_t[:], axis=mybir.AxisListType.X)
            rec_t = rowsp.tile([1, 1], F32, name="rec_t")
            nc.vector.reciprocal(rec_t[:], den_t[:])

            aout_t = outp.tile([1, S], F32, name="aout_t")
            nc.vector.tensor_scalar_mul(aout_t[:], rows_t[:], rec_t[:])
            nc.sync.dma_start(out=attn_d.ap()[b : b + 1, :], in_=aout_t[:])

            cs_t = outp.tile([1, H], F32, name="cs_t")
            nc.vector.tensor_scalar_mul(cs_t[0:1, 0:512], ctx_ps0[:], rec_t[:])
            nc.vector.tensor_scalar_mul(cs_t[0:1, 512:1024], ctx_ps1[:], rec_t[:])
            nc.sync.dma_start(out=ctx_d.ap()[b : b + 1, :], in_=cs_t[:])

    nc.compile()
    return nc


def _prep_inputs(hidden, enc, W_w, W_b, U_w, U_b, V_w):
    bf16 = ml_dtypes.bfloat16
    uwT = np.ascontiguousarray(U_w.T).astype(bf16)
    wwT = np.ascontiguousarray(W_w.T).astype(np.float32)
    wub = np.ascontiguousarray((W_b + U_b).reshape(MO, P).T).astype(np.float32)
    vcol = np.ascontiguousarray(V_w.reshape(MO, P).T).astype(np.float32)

    in_maps = []
    for i in range(NCORES):
        sl = slice(i * BL, (i + 1) * BL)
        e = enc[sl]
        in_maps.append(
            {
                "encT": np.ascontiguousarray(e.transpose(0, 2, 1)).astype(bf16),
                "encN": e.astype(bf16),
                "uwT": uwT,
                "wwT": wwT,
                "hidT": np.ascontiguousarray(hidden[sl, 0, :].T).astype(np.float32),
                "wub": wub,
                "vcol": vcol,
            }
        )
    return in_maps


def run(inputs: dict, trace: bool = False):
    """Build + run; returns ((context, attention), BassKernelResults)."""
    hidden = np.asarray(inputs["hidden"], dtype=np.float32)
    enc = np.asarray(inputs["encoder_output"], dtype=np.float32)
    W_w = np.asarray(inputs["W_w"], dtype=np.float32)
    W_b = np.asarray(inputs["W_b"], dtype=np.float32)
    U_w = np.asarray(inputs["U_w"], dtype=np.float32)
    U_b = np.asarray(inputs["U_b"], dtype=np.float32)
    V_w = np.asarray(inputs["V_w"], dtype=np.float32)
    V_b = np.asarray(inputs["V_b"], dtype=np.float32)

    nc = build_program(float(V_b.reshape(-1)[0]))
    in_maps = _prep_inputs(hidden, enc, W_w, W_b, U_w, U_b, V_w)
    res = run_bass_kernel_spmd(nc, in_maps, list(range(NCORES)), trace=trace)

    ctx = np.concatenate(
        [np.asarray(res.results[i]["ctx_out"]) for i in range(NCORES)], axis=0
    ).astype(np.float32)[:, None, :]
    attn = np.concatenate(
        [np.asarray(res.results[i]["attn_out"]) for i in range(NCORES)], axis=0
    ).astype(np.float32)[:, None, :]
    return (ctx, attn), res


def kernel(**inputs) -> tuple:
    out, _ = run(inputs, trace=False)
    return out


# revision 9
# speedup vs baseline: 1.3083x; 1.3083x over previous
"""Bahdanau attention on 8 Trainium2 NeuronCores.

Full inputs in, full outputs out. Batch (B=32) is sharded 4-per-core
(data parallel); all weights are replicated. Per core the kernel computes,
for each of its 4 batches:

    e_projT[k, s] = sum_h U_w[k, h] * enc[b, s, h]          (bf16 PE matmul)
    t[k, s]       = tanh(e_projT[k, s] + h_proj[b, k] + W_b[k] + U_b[k])
    scores[s]     = sum_k V[k] * t[k, s]                    (fp32 PE matmul)
    a[s]          = exp(scores[s] + V_b)                    (no max needed:
                      |scores| <= ||V||_1 + |V_b| ~ 26, exp fits fp32 easily)
    attention     = a / sum(a)
    context[h]    = sum_s a[s] * enc[b, s, h] / sum(a)      (bf16 PE matmul)

The h-contraction needs encoder tiles with h on partitions, while the
s-contraction needs s on partitions, so the host passes the encoder twice
(bf16 transposed + bf16 natural); 32 MB/core of DMA vs ~300 us of PE work.
"""

from contextlib import ExitStack

import numpy as np
import ml_dtypes

import concourse.bass as bass  # noqa: F401  (engine types resolve through bacc)
import concourse.mybir as mybir
from concourse import tile, bacc
from concourse.bass_utils import run_bass_kernel_spmd

BF16 = mybir.dt.bfloat16
F32 = mybir.dt.float32
AF = mybir.ActivationFunctionType

B, S, H = 32, 2048, 1024
NCORES = 8
BL = B // NCORES      # 4 batches per core
P = 128
KC = H // P           # 8 contraction chunks
MO = H // P           # 8 output-row chunks
SBLK = 4              # s-blocks per batch
SW = S // SBLK        # 512 columns per s-block
CH = SW // P          # 4 s-chunks of 128 per s-block
NQ = S // P           # 16 s-chunks per batch


def build_program(v_b: float) -> bacc.Bacc:
    nc = bacc.Bacc("TRN2", target_bir_lowering=False, debug=False, num_devices=NCORES)

    encT_d = nc.dram_tensor("encT", [BL, H, S], BF16, kind="ExternalInput")
    encN_d = nc.dram_tensor("encN", [BL, S, H], BF16, kind="ExternalInput")
    uwT_d = nc.dram_tensor("uwT", [H, H], BF16, kind="ExternalInput")
    wwT_d = nc.dram_tensor("wwT", [H, H], F32, kind="ExternalInput")
    hidT_d = nc.dram_tensor("hidT", [H, BL], F32, kind="ExternalInput")
    wub_d = nc.dram_tensor("wub", [P, MO], F32, kind="ExternalInput")
    vcol_d = nc.dram_tensor("vcol", [P, MO], BF16, kind="ExternalInput")
    ctx_d = nc.dram_tensor("ctx_out", [BL, H], F32, kind="ExternalOutput")
    attn_d = nc.dram_tensor("attn_out", [BL, S], F32, kind="ExternalOutput")

    with tile.TileContext(nc) as tc, ExitStack() as stack:
        const = stack.enter_context(tc.tile_pool(name="const", bufs=1))

        # Big weight + first encoder tiles stream on the SP (sync) HWDGE ring;
        # the fp32 W_w for h_proj streams in parallel on the ACT (scalar) ring
        # so it doesn't delay the main pipeline's first tiles.
        uw_t = const.tile([P, KC, H], BF16, name="uw_t")
        nc.sync.dma_start(
            out=uw_t[:], in_=uwT_d.ap().rearrange("(kc p) n -> p kc n", p=P)
        )
        hid_t = const.tile([P, KC, BL], F32, name="hid_t")
        nc.scalar.dma_start(
            out=hid_t[:], in_=hidT_d.ap().rearrange("(kc p) b -> p kc b", p=P)
        )
        wub_t = const.tile([P, MO], F32, name="wub_t")
        nc.scalar.dma_start(out=wub_t[:], in_=wub_d.ap())
        v_t = const.tile([P, MO], BF16, name="v_t")
        nc.scalar.dma_start(out=v_t[:], in_=vcol_d.ap())
        one_t = const.tile([1, 1], BF16, name="one_t")
        nc.vector.memset(one_t[:], 1.0)
        vb_t = const.tile([1, 1], F32, name="vb_t")
        nc.vector.memset(vb_t[:], v_b)
        ident_t = const.tile([P, P], F32, name="ident_t")
        from concourse.masks import make_identity

        make_identity(nc, ident_t)
        hb_t = const.tile([P, MO, BL], F32, name="hb_t")

        # ---- phase 0: h_proj[b, :] + (W_b + U_b), ending in column layout.
        # Wide fp32 matmuls (lhsT = hidden columns, so LDWEIGHTS is 4 cols),
        # then 8 small PE transposes to flip [4, 1024] rows into [128, 4]
        # per-chunk columns for the tanh bias.
        with (
            tc.tile_pool(name="wpool", bufs=1) as wpool,
            tc.tile_pool(name="hp_psum", bufs=2, space="PSUM") as hp_psum,
            tc.tile_pool(name="hrow_psum", bufs=2, space="PSUM") as hrow_psum,
        ):
            ww_t = wpool.tile([P, KC, H], F32, name="ww_t")
            ww_src = wwT_d.ap().rearrange("(kc p) n -> p kc n", p=P)
            for kc in range(KC):
                nc.scalar.dma_start(out=ww_t[:, kc, :], in_=ww_src[:, kc, :])
            hrow_ps = [
                hrow_psum.tile([BL, 512], F32, name=f"hrow_ps{h}") for h in range(2)
            ]
            for kc in range(KC):
                for h in range(2):
                    nc.tensor.matmul(
                        hrow_ps[h][:],
                        hid_t[:, kc, :],
                        ww_t[:, kc, h * 512 : (h + 1) * 512],
                        start=(kc == 0),
                        stop=(kc == KC - 1),
                    )
            hrow_t = wpool.tile([BL, H], F32, name="hrow_t")
            for h in range(2):
                nc.vector.tensor_copy(hrow_t[:, h * 512 : (h + 1) * 512], hrow_ps[h][:])
            for mo in range(MO):
                hcol_ps = hp_psum.tile([P, BL], F32, name="hcol_ps")
                nc.tensor.transpose(
                    hcol_ps[:], hrow_t[:, mo * P : (mo + 1) * P], ident_t[0:BL, 0:BL]
                )
                # hb[:, mo, b] = h_proj[b, mo*128+p] + (W_b + U_b)[mo*128+p]
                nc.scalar.activation(
                    hb_t[:, mo, :], hcol_ps[:], AF.Identity, bias=wub_t[:, mo : mo + 1]
                )

        # ---- main pools
        encTp = stack.enter_context(tc.tile_pool(name="encTp", bufs=2))
        encNp = stack.enter_context(tc.tile_pool(name="encNp", bufs=2))
        tanhp = stack.enter_context(tc.tile_pool(name="tanhp", bufs=3))
        rowsp = stack.enter_context(tc.tile_pool(name="rowsp", bufs=2))
        acolp = stack.enter_context(tc.tile_pool(name="acolp", bufs=2))
        outp = stack.enter_context(tc.tile_pool(name="outp", bufs=2))
        ep_psum = stack.enter_context(tc.tile_pool(name="ep_psum", bufs=2, space="PSUM"))
        sc_psum = stack.enter_context(tc.tile_pool(name="sc_psum", bufs=2, space="PSUM"))
        at_psum = stack.enter_context(tc.tile_pool(name="at_psum", bufs=2, space="PSUM"))
        ctx_psum = stack.enter_context(tc.tile_pool(name="ctx_psum", bufs=1, space="PSUM"))

        for b in range(BL):
            rows_t = rowsp.tile([1, S], BF16, name="rows_t")
            den4_t = rowsp.tile([1, SBLK], F32, name="den4_t")
            ctx_ps0 = ctx_psum.tile([1, 512], F32, name="ctx_ps0")
            ctx_ps1 = ctx_psum.tile([1, 512], F32, name="ctx_ps1")

            for sb in range(SBLK):
                encT_t = encTp.tile([P, KC, SW], BF16, name="encT_t")
                nc.sync.dma_start(
                    out=encT_t[:],
                    in_=encT_d.ap()[b].rearrange("(kc p) s -> p kc s", p=P)[
                        :, :, sb * SW : (sb + 1) * SW
                    ],
                )
                encN_t = encNp.tile([P, CH, H], BF16, name="encN_t")
                nc.sync.dma_start(
                    out=encN_t[:],
                    in_=encN_d.ap()[b].rearrange("(c p) h -> p c h", p=P)[
                        :, sb * CH : (sb + 1) * CH, :
                    ],
                )

                sc_ps = sc_psum.tile([1, SW], F32, name="sc_ps")
                tanh_tiles = []
                # e_projT + tanh, with the V-dot staggered one mo behind so the
                # PE never stalls waiting on ACT.
                for mo in range(MO):
                    ep_ps = ep_psum.tile([P, SW], F32, name="ep_ps")
                    for kc in range(KC):
                        nc.tensor.matmul(
                            ep_ps[:],
                            uw_t[:, kc, mo * P : (mo + 1) * P],
                            encT_t[:, kc, :],
                            start=(kc == 0),
                            stop=(kc == KC - 1),
                        )
                    tanh_t = tanhp.tile([P, SW], BF16, name="tanh_t")
                    nc.scalar.activation(
                        tanh_t[:], ep_ps[:], AF.Tanh, bias=hb_t[:, mo, b : b + 1]
                    )
                    tanh_tiles.append(tanh_t)
                    if mo >= 1:
                        nc.tensor.matmul(
                            sc_ps[:],
                            v_t[:, mo - 1 : mo],
                            tanh_tiles[mo - 1][:],
                            start=(mo - 1 == 0),
                            stop=False,
                            skip_group_check=True,
                        )
                nc.tensor.matmul(
                    sc_ps[:],
                    v_t[:, MO - 1 : MO],
                    tanh_tiles[MO - 1][:],
                    start=False,
                    stop=True,
                    skip_group_check=True,
                )

                # a = exp(scores + V_b); accum_out gives this block's sum(a)
                nc.scalar.activation(
                    rows_t[0:1, sb * SW : (sb + 1) * SW],
                    sc_ps[:],
                    AF.Exp,
                    bias=vb_t[:],
                    accum_out=den4_t[0:1, sb : sb + 1],
                )

                # turn the a-row into 128-deep columns: K=1 matmul against [[1.0]]
                acol_t = acolp.tile([P, CH], BF16, name="acol_t")
                for c in range(CH):
                    q = sb * CH + c
                    at_ps = at_psum.tile([P, 1], F32, name="at_ps")
                    nc.tensor.matmul(
                        at_ps[:],
                        rows_t[0:1, q * P : (q + 1) * P],
                        one_t[:],
                        start=True,
                        stop=True,
                        skip_group_check=True,
                    )
                    nc.vector.tensor_copy(acol_t[:, c : c + 1], at_ps[:])

                # context accumulation over all 16 s-chunks of this batch
                for c in range(CH):
                    q = sb * CH + c
                    st = q == 0
                    sp = q == NQ - 1
                    nc.tensor.matmul(
                        ctx_ps0[:],
                        acol_t[:, c : c + 1],
                        encN_t[:, c, 0:512],
                        start=st,
                        stop=sp,
                        skip_group_check=True,
                    )
                    nc.tensor.matmul(
                        ctx_ps1[:],
                        acol_t[:, c : c + 1],
                        encN_t[:, c, 512:1024],
                        start=st,
                        stop=sp,
                        skip_group_check=True,
                    )

            den_t = rowsp.tile([1, 1], F32, name="den_t")
            nc.vector.reduce_sum(den_t[:], den4_t[:], axis=mybir.AxisListType.X)
            rec_t = rowsp.tile([1, 1], F32, name="rec_t")
            nc.vector.reciprocal(rec_t[:], den_t[:])

            aout_t = outp.tile([1, S], F32, name="aout_t")
            nc.vector.tensor_scalar_mul(aout_t[:], rows_t[:], rec_t[:])
            nc.sync.dma_start(out=attn_d.ap()[b : b + 1, :], in_=aout_t[:])

            cs_t = outp.tile([1, H], F32, name="cs_t")
            nc.vector.tensor_scalar_mul(cs_t[0:1, 0:512], ctx_ps0[:], rec_t[:])
            nc.vector.tensor_scalar_mul(cs_t[0:1, 512:1024], ctx_ps1[:], rec_t[:])
            nc.sync.dma_start(out=ctx_d.ap()[b : b + 1, :], in_=cs_t[:])

    nc.compile()
    return nc


def _prep_inputs(hidden, enc, W_w, W_b, U_w, U_b, V_w):
    bf16 = ml_dtypes.bfloat16
    uwT = np.ascontiguousarray(U_w.T).astype(bf16)
    wwT = np.ascontiguousarray(W_w.T).astype(np.float32)
    wub = np.ascontiguousarray((W_b + U_b).reshape(MO, P).T).astype(np.float32)
    vcol = np.ascontiguousarray(V_w.reshape(MO, P).T).astype(ml_dtypes.bfloat16)

    in_maps = []
    for i in range(NCORES):
        sl = slice(i * BL, (i + 1) * BL)
        e = enc[sl]
        in_maps.append(
            {
                "encT": np.ascontiguousarray(e.transpose(0, 2, 1)).astype(bf16),
                "encN": e.astype(bf16),
                "uwT": uwT,
                "wwT": wwT,
                "hidT": np.ascontiguousarray(hidden[sl, 0, :].T).astype(np.float32),
                "wub": wub,
                "vcol": vcol,
            }
        )
    return in_maps


def run(inputs: dict, trace: bool = False):
    """Build + run; returns ((context, attention), BassKernelResults)."""
    hidden = np.asarray(inputs["hidden"], dtype=np.float32)
    enc = np.asarray(inputs["encoder_output"], dtype=np.float32)
    W_w = np.asarray(inputs["W_w"], dtype=np.float32)
    W_b = np.asarray(inputs["W_b"], dtype=np.float32)
    U_w = np.asarray(inputs["U_w"], dtype=np.float32)
    U_b = np.asarray(inputs["U_b"], dtype=np.float32)
    V_w = np.asarray(inputs["V_w"], dtype=np.float32)
    V_b = np.asarray(inputs["V_b"], dtype=np.float32)

    nc = build_program(float(V_b.reshape(-1)[0]))
    in_maps = _prep_inputs(hidden, enc, W_w, W_b, U_w, U_b, V_w)
    res = run_bass_kernel_spmd(nc, in_maps, list(range(NCORES)), trace=trace)

    ctx = np.concatenate(
        [np.asarray(res.results[i]["ctx_out"]) for i in range(NCORES)], axis=0
    ).astype(np.float32)[:, None, :]
    attn = np.concatenate(
        [np.asarray(res.results[i]["attn_out"]) for i in range(NCORES)], axis=0
    ).astype(np.float32)[:, None, :]
    return (ctx, attn), res


def kernel(**inputs) -> tuple:
    out, _ = run(inputs, trace=False)
    return out


# revision 12
# speedup vs baseline: 1.3422x; 1.0259x over previous
"""Bahdanau attention on 8 Trainium2 NeuronCores.

Full inputs in, full outputs out. Batch (B=32) is sharded 4-per-core
(data parallel); all weights are replicated. Per core the kernel computes,
for each of its 4 batches:

    e_projT[k, s] = sum_h U_w[k, h] * enc[b, s, h]          (bf16 PE matmul)
    t[k, s]       = tanh(e_projT[k, s] + h_proj[b, k] + W_b[k] + U_b[k])
    scores[s]     = sum_k V[k] * t[k, s]                    (fp32 PE matmul)
    a[s]          = exp(scores[s] + V_b)                    (no max needed:
                      |scores| <= ||V||_1 + |V_b| ~ 26, exp fits fp32 easily)
    attention     = a / sum(a)
    context[h]    = sum_s a[s] * enc[b, s, h] / sum(a)      (bf16 PE matmul)

The h-contraction needs encoder tiles with h on partitions, while the
s-contraction needs s on partitions, so the host passes the encoder twice
(bf16 transposed + bf16 natural); 32 MB/core of DMA vs ~300 us of PE work.
"""

from contextlib import ExitStack

import numpy as np
import ml_dtypes

import concourse.bass as bass  # noqa: F401  (engine types resolve through bacc)
import concourse.mybir as mybir
from concourse import tile, bacc
from concourse.bass_utils import run_bass_kernel_spmd

BF16 = mybir.dt.bfloat16
F32 = mybir.dt.float32
AF = mybir.ActivationFunctionType

B, S, H = 32, 2048, 1024
NCORES = 8
BL = B // NCORES      # 4 batches per core
P = 128
KC = H // P           # 8 contraction chunks
MO = H // P           # 8 output-row chunks
SBLK = 4              # s-blocks per batch
SW = S // SBLK        # 512 columns per s-block
CH = SW // P          # 4 s-chunks of 128 per s-block
NQ = S // P           # 16 s-chunks per batch


def build_program(v_b: float) -> bacc.Bacc:
    nc = bacc.Bacc("TRN2", target_bir_lowering=False, debug=False, num_devices=NCORES)

    encT_d = nc.dram_tensor("encT", [BL, H, S], BF16, kind="ExternalInput")
    encN_d = nc.dram_tensor("encN", [BL, S, H], BF16, kind="ExternalInput")
    uwT_d = nc.dram_tensor("uwT", [H, H], BF16, kind="ExternalInput")
    wwT_d = nc.dram_tensor("wwT", [H, H], F32, kind="ExternalInput")
    hidT_d = nc.dram_tensor("hidT", [H, BL], F32, kind="ExternalInput")
    wub_d = nc.dram_tensor("wub", [P, MO], F32, kind="ExternalInput")
    vcol_d = nc.dram_tensor("vcol", [P, MO], BF16, kind="ExternalInput")
    ctx_d = nc.dram_tensor("ctx_out", [BL, H], F32, kind="ExternalOutput")
    attn_d = nc.dram_tensor("attn_out", [BL, S], F32, kind="ExternalOutput")

    with tile.TileContext(nc) as tc, ExitStack() as stack:
        const = stack.enter_context(tc.tile_pool(name="const", bufs=1))

        # Big weight + first encoder tiles stream on the SP (sync) HWDGE ring;
        # the fp32 W_w for h_proj streams in parallel on the ACT (scalar) ring
        # so it doesn't delay the main pipeline's first tiles.
        uw_t = const.tile([P, KC, H], BF16, name="uw_t")
        nc.sync.dma_start(
            out=uw_t[:], in_=uwT_d.ap().rearrange("(kc p) n -> p kc n", p=P)
        )
        hid_t = const.tile([P, KC, BL], F32, name="hid_t")
        nc.scalar.dma_start(
            out=hid_t[:], in_=hidT_d.ap().rearrange("(kc p) b -> p kc b", p=P)
        )
        wub_t = const.tile([P, MO], F32, name="wub_t")
        nc.scalar.dma_start(out=wub_t[:], in_=wub_d.ap())
        v_t = const.tile([P, MO], BF16, name="v_t")
        nc.scalar.dma_start(out=v_t[:], in_=vcol_d.ap())
        one_t = const.tile([1, 1], BF16, name="one_t")
        nc.vector.memset(one_t[:], 1.0)
        vb_t = const.tile([1, 1], F32, name="vb_t")
        nc.vector.memset(vb_t[:], v_b)
        ident_t = const.tile([P, P], F32, name="ident_t")
        from concourse.masks import make_identity

        make_identity(nc, ident_t)
        hb_t = const.tile([P, MO, BL], F32, name="hb_t")

        # ---- phase 0: h_proj[b, :] + (W_b + U_b), ending in column layout.
        # Wide fp32 matmuls (lhsT = hidden columns, so LDWEIGHTS is 4 cols),
        # then 8 small PE transposes to flip [4, 1024] rows into [128, 4]
        # per-chunk columns for the tanh bias. Emitted *between* the first two
        # eproj groups of batch 0 so the PE warms up and the encoder DMAs
        # overlap it; its PSUM pools use banks disjoint from ep_psum.
        def emit_hproj():
            with (
                tc.tile_pool(name="wpool", bufs=1) as wpool,
                tc.tile_pool(name="hp_psum", bufs=2, space="PSUM") as hp_psum,
                tc.tile_pool(name="hrow_psum", bufs=1, space="PSUM") as hrow_psum,
            ):
                ww_t = wpool.tile([P, KC, H], F32, name="ww_t")
                ww_src = wwT_d.ap().rearrange("(kc p) n -> p kc n", p=P)
                for kc in range(KC):
                    nc.scalar.dma_start(out=ww_t[:, kc, :], in_=ww_src[:, kc, :])
                hrow_ps = [
                    hrow_psum.tile([BL, 512], F32, name=f"hrow_ps{h}") for h in range(2)
                ]
                for kc in range(KC):
                    for h in range(2):
                        nc.tensor.matmul(
                            hrow_ps[h][:],
                            hid_t[:, kc, :],
                            ww_t[:, kc, h * 512 : (h + 1) * 512],
                            start=(kc == 0),
                            stop=(kc == KC - 1),
                        )
                hrow_t = wpool.tile([BL, H], F32, name="hrow_t")
                for h in range(2):
                    nc.vector.tensor_copy(
                        hrow_t[:, h * 512 : (h + 1) * 512], hrow_ps[h][:]
                    )
                for mo in range(MO):
                    hcol_ps = hp_psum.tile([P, BL], F32, name="hcol_ps")
                    nc.tensor.transpose(
                        hcol_ps[:], hrow_t[:, mo * P : (mo + 1) * P], ident_t[0:BL, 0:BL]
                    )
                    # hb[:, mo, b] = h_proj[b, mo*128+p] + (W_b + U_b)[mo*128+p]
                    nc.scalar.activation(
                        hb_t[:, mo, :],
                        hcol_ps[:],
                        AF.Identity,
                        bias=wub_t[:, mo : mo + 1],
                    )

        # ---- main pools (ep_psum first: its banks must be disjoint from the
        # phase-0 psum pools so batch 0's eproj overlaps h_proj)
        encTp = stack.enter_context(tc.tile_pool(name="encTp", bufs=2))
        encNp = stack.enter_context(tc.tile_pool(name="encNp", bufs=2))
        tanhp = stack.enter_context(tc.tile_pool(name="tanhp", bufs=3))
        rowsp = stack.enter_context(tc.tile_pool(name="rowsp", bufs=2))
        acolp = stack.enter_context(tc.tile_pool(name="acolp", bufs=2))
        outp = stack.enter_context(tc.tile_pool(name="outp", bufs=2))
        ep_psum = stack.enter_context(tc.tile_pool(name="ep_psum", bufs=2, space="PSUM"))
        late = {}

        def late_pools():
            # allocated after the phase-0 pools release, reusing their banks
            late["sc"] = stack.enter_context(
                tc.tile_pool(name="sc_psum", bufs=2, space="PSUM")
            )
            late["at"] = stack.enter_context(
                tc.tile_pool(name="at_psum", bufs=2, space="PSUM")
            )
            late["ctx"] = stack.enter_context(
                tc.tile_pool(name="ctx_psum", bufs=1, space="PSUM")
            )

        for b in range(BL):
            rows_t = rowsp.tile([1, S], BF16, name="rows_t")
            den4_t = rowsp.tile([1, SBLK], F32, name="den4_t")
            ctx_ps0 = ctx_ps1 = None  # allocated after late_pools()

            for sb in range(SBLK):
                first = b == 0 and sb == 0
                encT_t = encTp.tile([P, KC, SW], BF16, name="encT_t")
                nc.sync.dma_start(
                    out=encT_t[:],
                    in_=encT_d.ap()[b].rearrange("(kc p) s -> p kc s", p=P)[
                        :, :, sb * SW : (sb + 1) * SW
                    ],
                )
                encN_t = encNp.tile([P, CH, H], BF16, name="encN_t")
                nc.sync.dma_start(
                    out=encN_t[:],
                    in_=encN_d.ap()[b].rearrange("(c p) h -> p c h", p=P)[
                        :, sb * CH : (sb + 1) * CH, :
                    ],
                )

                tanh_tiles = []
                vdot_emitted = 0

                def emit_ep(mo):
                    ep_ps = ep_psum.tile([P, SW], F32, name="ep_ps")
                    for kc in range(KC):
                        nc.tensor.matmul(
                            ep_ps[:],
                            uw_t[:, kc, mo * P : (mo + 1) * P],
                            encT_t[:, kc, :],
                            start=(kc == 0),
                            stop=(kc == KC - 1),
                        )
                    return ep_ps

                def emit_tanh(mo, ep_ps):
                    tanh_t = tanhp.tile([P, SW], BF16, name="tanh_t")
                    nc.scalar.activation(
                        tanh_t[:], ep_ps[:], AF.Tanh, bias=hb_t[:, mo, b : b + 1]
                    )
                    tanh_tiles.append(tanh_t)

                def emit_vdot(mo):
                    nc.tensor.matmul(
                        sc_ps[:],
                        v_t[:, mo : mo + 1],
                        tanh_tiles[mo][:],
                        start=(mo == 0),
                        stop=(mo == MO - 1),
                        skip_group_check=True,
                    )

                # e_projT + tanh, with the V-dot staggered behind so the PE
                # never stalls waiting on ACT. On the very first block, slot
                # h_proj in after two eproj groups (PE is warm, DMAs overlap).
                if first:
                    ep0 = emit_ep(0)
                    ep1 = emit_ep(1)
                    emit_hproj()
                    late_pools()
                    sc_ps = late["sc"].tile([1, SW], F32, name="sc_ps")
                    emit_tanh(0, ep0)
                    emit_tanh(1, ep1)
                    for mo in range(2, MO):
                        ep_ps = emit_ep(mo)
                        emit_tanh(mo, ep_ps)
                        emit_vdot(mo - 2)
                    emit_vdot(MO - 2)
                    emit_vdot(MO - 1)
                else:
                    sc_ps = late["sc"].tile([1, SW], F32, name="sc_ps")
                    for mo in range(MO):
                        ep_ps = emit_ep(mo)
                        emit_tanh(mo, ep_ps)
                        if mo >= 1:
                            emit_vdot(mo - 1)
                    emit_vdot(MO - 1)

                if ctx_ps0 is None:
                    ctx_ps0 = late["ctx"].tile([1, 512], F32, name="ctx_ps0")
                    ctx_ps1 = late["ctx"].tile([1, 512], F32, name="ctx_ps1")

                # a = exp(scores + V_b); accum_out gives this block's sum(a)
                nc.scalar.activation(
                    rows_t[0:1, sb * SW : (sb + 1) * SW],
                    sc_ps[:],
                    AF.Exp,
                    bias=vb_t[:],
                    accum_out=den4_t[0:1, sb : sb + 1],
                )

                # turn the a-row into 128-deep columns: K=1 matmul against [[1.0]]
                acol_t = acolp.tile([P, CH], BF16, name="acol_t")
                for c in range(CH):
                    q = sb * CH + c
                    at_ps = late["at"].tile([P, 1], F32, name="at_ps")
                    nc.tensor.matmul(
                        at_ps[:],
                        rows_t[0:1, q * P : (q + 1) * P],
                        one_t[:],
                        start=True,
                        stop=True,
                        skip_group_check=True,
                    )
                    nc.vector.tensor_copy(acol_t[:, c : c + 1], at_ps[:])

                # context accumulation over all 16 s-chunks of this batch
                for c in range(CH):
                    q = sb * CH + c
                    st = q == 0
                    sp = q == NQ - 1
                    nc.tensor.matmul(
                        ctx_ps0[:],
                        acol_t[:, c : c + 1],
                        encN_t[:, c, 0:512],
                        start=st,
                        stop=sp,
                        skip_group_check=True,
                    )
                    nc.tensor.matmul(
                        ctx_ps1[:],
                        acol_t[:, c : c + 1],
                        encN_t[:, c, 512:1024],
                        start=st,
                        stop=sp,
                        skip_group_check=True,
                    )

            den_t = rowsp.tile([1, 1], F32, name="den_t")
            nc.vector.reduce_sum(den_t[:], den4_t[:], axis=mybir.AxisListType.X)
            rec_t = rowsp.tile([1, 1], F32, name="rec_t")
            nc.vector.reciprocal(rec_t[:], den_t[:])

            aout_t = outp.tile([1, S], F32, name="aout_t")
            nc.vector.tensor_scalar_mul(aout_t[:], rows_t[:], rec_t[:])
            nc.sync.dma_start(out=attn_d.ap()[b : b + 1, :], in_=aout_t[:])

            cs_t = outp.tile([1, H], F32, name="cs_t")
            nc.vector.tensor_scalar_mul(cs_t[0:1, 0:512], ctx_ps0[:], rec_t[:])
            nc.vector.tensor_scalar_mul(cs_t[0:1, 512:1024], ctx_ps1[:], rec_t[:])
            nc.sync.dma_start(out=ctx_d.ap()[b : b + 1, :], in_=cs_t[:])

    nc.compile()
    return nc


def _prep_inputs(hidden, enc, W_w, W_b, U_w, U_b, V_w):
    bf16 = ml_dtypes.bfloat16
    uwT = np.ascontiguousarray(U_w.T).astype(bf16)
    wwT = np.ascontiguousarray(W_w.T).astype(np.float32)
    wub = np.ascontiguousarray((W_b + U_b).reshape(MO, P).T).astype(np.float32)
    vcol = np.ascontiguousarray(V_w.reshape(MO, P).T).astype(ml_dtypes.bfloat16)

    in_maps = []
    for i in range(NCORES):
        sl = slice(i * BL, (i + 1) * BL)
        e = enc[sl]
        in_maps.append(
            {
                "encT": np.ascontiguousarray(e.transpose(0, 2, 1)).astype(bf16),
                "encN": e.astype(bf16),
                "uwT": uwT,
                "wwT": wwT,
                "hidT": np.ascontiguousarray(hidden[sl, 0, :].T).astype(np.float32),
                "wub": wub,
                "vcol": vcol,
            }
        )
    return in_maps


def run(inputs: dict, trace: bool = False):
    """Build + run; returns ((context, attention), BassKernelResults)."""
    hidden = np.asarray(inputs["hidden"], dtype=np.float32)
    enc = np.asarray(inputs["encoder_output"], dtype=np.float32)
    W_w = np.asarray(inputs["W_w"], dtype=np.float32)
    W_b = np.asarray(inputs["W_b"], dtype=np.float32)
    U_w = np.asarray(inputs["U_w"], dtype=np.float32)
    U_b = np.asarray(inputs["U_b"], dtype=np.float32)
    V_w = np.asarray(inputs["V_w"], dtype=np.float32)
    V_b = np.asarray(inputs["V_b"], dtype=np.float32)

    nc = build_program(float(V_b.reshape(-1)[0]))
    in_maps = _prep_inputs(hidden, enc, W_w, W_b, U_w, U_b, V_w)
    res = run_bass_kernel_spmd(nc, in_maps, list(range(NCORES)), trace=trace)

    ctx = np.concatenate(
        [np.asarray(res.results[i]["ctx_out"]) for i in range(NCORES)], axis=0
    ).astype(np.float32)[:, None, :]
    attn = np.concatenate(
        [np.asarray(res.results[i]["attn_out"]) for i in range(NCORES)], axis=0
    ).astype(np.float32)[:, None, :]
    return (ctx, attn), res


def kernel(**inputs) -> tuple:
    out, _ = run(inputs, trace=False)
    return out


# revision 21
# speedup vs baseline: 1.4678x; 1.0936x over previous
"""Bahdanau attention on 8 Trainium2 NeuronCores.

Full inputs in, full outputs out. Batch (B=32) is sharded 4-per-core
(data parallel); all weights are replicated. Per core, for each of its
4 batches:

    e_proj[s, k]  = sum_h enc[b, s, h] * U_w[k, h]          (bf16 PE matmul,
                     s on partitions, k on the free axis)
    t[s, k]       = tanh(e_proj[s, k] + h_proj[b, k] + W_b[k] + U_b[k])
                     (DVE add of the broadcast row bias, then ACT tanh)
    scores[s]     = sum_k V[k] * t[s, k]                     (DVE fused
                     multiply + free-axis reduce — keeps the PE free)
    a[s]          = exp(scores[s] + V_b)                     (no max needed:
                     |scores| <= ||V||_1 + |V_b| ~ 26, exp fits fp32 easily)
    attention     = a / sum(a)        (sum via tiny ones-matmuls in PSUM)
    context[h]    = sum_s a[s] * enc[b, s, h] / sum(a)       (bf16 PE matmul)

The h-contraction needs encoder tiles with h on partitions while the
s-contraction needs s on partitions, so the host passes the encoder twice
(bf16 transposed + bf16 natural); 32 MB/core of DMA under ~280 us of PE work.
Scores emerge as 128-deep columns, which is exactly the lhsT layout the
context matmul needs — no on-chip transposes anywhere.
"""

from contextlib import ExitStack

import numpy as np
import ml_dtypes

import concourse.bass as bass  # noqa: F401
import concourse.mybir as mybir
from concourse import tile, bacc
from concourse.bass_utils import run_bass_kernel_spmd
from concourse.bass_isa import ReduceOp  # noqa: F401

BF16 = mybir.dt.bfloat16
F32 = mybir.dt.float32
AF = mybir.ActivationFunctionType
ALU = mybir.AluOpType

B, S, H = 32, 2048, 1024
NCORES = 8
BL = B // NCORES      # 4 batches per core
P = 128
KC = H // P           # 8 contraction chunks
SBLK = 4              # s-blocks per batch (DMA granularity)
SW = S // SBLK        # 512 rows per s-block
CH = SW // P          # 4 s-chunks of 128 per s-block
NQ = S // P           # 16 s-chunks per batch


def build_program(v_b: float) -> bacc.Bacc:
    nc = bacc.Bacc("TRN2", target_bir_lowering=False, debug=False, num_devices=NCORES)

    encT_d = nc.dram_tensor("encT", [BL, H, S], BF16, kind="ExternalInput")
    encN_d = nc.dram_tensor("encN", [BL, S, H], BF16, kind="ExternalInput")
    uwT_d = nc.dram_tensor("uwT", [H, H], BF16, kind="ExternalInput")
    wwT_d = nc.dram_tensor("wwT", [H, H], F32, kind="ExternalInput")
    hidT_d = nc.dram_tensor("hidT", [H, BL], F32, kind="ExternalInput")
    wub4_d = nc.dram_tensor("wub4", [BL, H], F32, kind="ExternalInput")
    v128_d = nc.dram_tensor("v128", [P, H], BF16, kind="ExternalInput")
    ctx_d = nc.dram_tensor("ctx_out", [BL, H], F32, kind="ExternalOutput")
    attn_d = nc.dram_tensor("attn_out", [BL, S], F32, kind="ExternalOutput")
    hrow_dram = nc.dram_tensor("hrow_scratch", [BL, H], F32)
    rec_dram = nc.dram_tensor("rec_scratch", [BL, 1], F32)

    with tile.TileContext(nc) as tc, ExitStack() as stack:
        const = stack.enter_context(tc.tile_pool(name="const", bufs=1))

        # sync (SP) HWDGE ring: the big bf16 operands the main loop needs first
        uw_t = const.tile([P, KC, H], BF16, name="uw_t")
        nc.sync.dma_start(
            out=uw_t[:], in_=uwT_d.ap().rearrange("(kc p) n -> p kc n", p=P)
        )
        # scalar (ACT) HWDGE ring: everything h_proj needs, in parallel
        hid_t = const.tile([P, KC, BL], F32, name="hid_t")
        nc.scalar.dma_start(
            out=hid_t[:], in_=hidT_d.ap().rearrange("(kc p) b -> p kc b", p=P)
        )
        wub4_t = const.tile([BL, H], F32, name="wub4_t")
        nc.scalar.dma_start(out=wub4_t[:], in_=wub4_d.ap())
        v128_t = const.tile([P, H], BF16, name="v128_t")
        nc.scalar.dma_start(out=v128_t[:], in_=v128_d.ap())
        one_t = const.tile([P, 1], BF16, name="one_t")
        nc.vector.memset(one_t[:], 1.0)
        vb_t = const.tile([P, 1], F32, name="vb_t")
        nc.vector.memset(vb_t[:], v_b)
        # hbrow[b, k] = h_proj[b, k] + W_b[k] + U_b[k]; hb128[.] broadcasts
        # the current batch's row across all 128 partitions (gpsimd).
        hbrow_t = const.tile([BL, H], F32, name="hbrow_t")

        # ---- phase 0: h_proj rows. lhsT = hidden columns (LDWEIGHTS is only
        # 4 cols) so this is 16 wide fp32 matmuls, nothing else.
        def emit_hproj():
            with (
                tc.tile_pool(name="wpool", bufs=1) as wpool,
                tc.tile_pool(name="hrow_psum", bufs=1, space="PSUM") as hrow_psum,
            ):
                ww_t = wpool.tile([P, KC, H], F32, name="ww_t")
                ww_src = wwT_d.ap().rearrange("(kc p) n -> p kc n", p=P)
                for kc in range(KC):
                    nc.scalar.dma_start(out=ww_t[:, kc, :], in_=ww_src[:, kc, :])
                hrow_ps = [
                    hrow_psum.tile([BL, 512], F32, name=f"hrow_ps{h}") for h in range(2)
                ]
                for kc in range(KC):
                    for h in range(2):
                        nc.tensor.matmul(
                            hrow_ps[h][:],
                            hid_t[:, kc, :],
                            ww_t[:, kc, h * 512 : (h + 1) * 512],
                            start=(kc == 0),
                            stop=(kc == KC - 1),
                        )
                for h in range(2):
                    nc.vector.tensor_add(
                        hbrow_t[:, h * 512 : (h + 1) * 512],
                        hrow_ps[h][:],
                        wub4_t[:, h * 512 : (h + 1) * 512],
                    )

        # ---- main pools (ep_psum first: disjoint banks from phase-0 pools)
        encTp = stack.enter_context(tc.tile_pool(name="encTp", bufs=2))
        encNp = stack.enter_context(tc.tile_pool(name="encNp", bufs=2))
        esp = stack.enter_context(tc.tile_pool(name="esp", bufs=3))
        tanhp = stack.enter_context(tc.tile_pool(name="tanhp", bufs=3))
        scrp = stack.enter_context(tc.tile_pool(name="scrp", bufs=2))
        accp = stack.enter_context(tc.tile_pool(name="accp", bufs=3))
        acolp = stack.enter_context(tc.tile_pool(name="acolp", bufs=2))
        hbp = stack.enter_context(tc.tile_pool(name="hbp", bufs=2))
        outp = stack.enter_context(tc.tile_pool(name="outp", bufs=2))
        ep_psum = stack.enter_context(tc.tile_pool(name="ep_psum", bufs=2, space="PSUM"))
        late = {}

        def late_pools():
            late["ctx"] = stack.enter_context(
                tc.tile_pool(name="ctx_psum", bufs=1, space="PSUM")
            )
            late["den"] = stack.enter_context(
                tc.tile_pool(name="den_psum", bufs=1, space="PSUM")
            )

        pending = []  # deferred PE work (ctx/den matmuls), staggered 2 chunks

        def drain_pending(keep=2):
            while len(pending) > keep:
                pending.pop(0)()

        emit_hproj()
        late_pools()

        for b in range(BL):
            # replicate this batch's bias row to all partitions via DRAM
            nc.sync.dma_start(out=hrow_dram.ap()[b : b + 1, :], in_=hbrow_t[b : b + 1, :])
            hb128_t = hbp.tile([P, H], F32, name="hb128_t")
            nc.sync.dma_start(
                out=hb128_t[:], in_=hrow_dram.ap()[b].partition_broadcast(P)
            )
            acol_t = acolp.tile([P, NQ], BF16, name="acol_t")
            cps = {}

            for sb in range(SBLK):
                encT_t = encTp.tile([P, KC, SW], BF16, name="encT_t")
                nc.sync.dma_start(
                    out=encT_t[:],
                    in_=encT_d.ap()[b].rearrange("(kc p) s -> p kc s", p=P)[
                        :, :, sb * SW : (sb + 1) * SW
                    ],
                )
                encN_t = encNp.tile([P, CH, H], BF16, name="encN_t")
                nc.sync.dma_start(
                    out=encN_t[:],
                    in_=encN_d.ap()[b].rearrange("(c p) h -> p c h", p=P)[
                        :, sb * CH : (sb + 1) * CH, :
                    ],
                )

                for c in range(CH):
                    q = sb * CH + c
                    # e_proj for 128 s-rows: [128, 1024] over two psum banks
                    ep_ps = ep_psum.tile([P, H], F32, name="ep_ps")
                    for kc in range(KC):
                        for h in range(2):
                            nc.tensor.matmul(
                                ep_ps[:, h * 512 : (h + 1) * 512],
                                encT_t[:, kc, c * P : (c + 1) * P],
                                uw_t[:, kc, h * 512 : (h + 1) * 512],
                                start=(kc == 0),
                                stop=(kc == KC - 1),
                            )

                    # + row bias, tanh, V-weighted free-axis reduce -> scores col
                    es_t = esp.tile([P, H], F32, name="es_t")
                    nc.vector.tensor_add(es_t[:], ep_ps[:], hb128_t[:])
                    tanh_t = tanhp.tile([P, H], BF16, name="tanh_t")
                    nc.scalar.activation(tanh_t[:], es_t[:], AF.Tanh)
                    scr_t = scrp.tile([P, H], BF16, name="scr_t")
                    nc.vector.tensor_mul(scr_t[:], tanh_t[:], v128_t[:])
                    acc_t = accp.tile([P, 1], F32, name="acc_t")
                    nc.vector.reduce_sum(acc_t[:], scr_t[:], axis=mybir.AxisListType.X)
                    # a-column = exp(scores + V_b), straight into lhsT layout
                    nc.scalar.activation(
                        acol_t[:, q : q + 1], acc_t[:], AF.Exp, bias=vb_t[:]
                    )

                    def emit_ctx(q=q, encN_t=encN_t, c=c):
                        if not cps:
                            cps["c0"] = late["ctx"].tile([1, 512], F32, name="ctx_ps0")
                            cps["c1"] = late["ctx"].tile([1, 512], F32, name="ctx_ps1")
                            cps["d"] = late["den"].tile([1, 1], F32, name="den_ps")
                        st = q == 0
                        sp = q == NQ - 1
                        nc.tensor.matmul(
                            cps["c0"][:], acol_t[:, q : q + 1], encN_t[:, c, 0:512],
                            start=st, stop=sp, skip_group_check=True,
                        )
                        nc.tensor.matmul(
                            cps["c1"][:], acol_t[:, q : q + 1], encN_t[:, c, 512:1024],
                            start=st, stop=sp, skip_group_check=True,
                        )
                        nc.tensor.matmul(
                            cps["d"][:], acol_t[:, q : q + 1], one_t[:],
                            start=st, stop=sp, skip_group_check=True,
                        )

                    pending.append(emit_ctx)
                    drain_pending(keep=2)

            drain_pending(keep=0)  # batch boundary: finish ctx/den for b
            ctx_ps0, ctx_ps1, den_ps = cps["c0"], cps["c1"], cps["d"]

            den_t = outp.tile([1, 1], F32, name="den_t")
            rec_t = outp.tile([1, 1], F32, name="rec_t")
            nc.vector.tensor_copy(den_t[:], den_ps[:])
            nc.vector.reciprocal(rec_t[:], den_t[:])
            nc.sync.dma_start(out=rec_dram.ap()[b : b + 1, :], in_=rec_t[:])
            rec128_t = accp.tile([P, 1], F32, name="rec128_t")
            nc.sync.dma_start(
                out=rec128_t[:], in_=rec_dram.ap()[b].partition_broadcast(P)
            )

            acoln_t = outp.tile([P, NQ], F32, name="acoln_t")
            nc.vector.tensor_scalar_mul(acoln_t[:], acol_t[:], rec128_t[:])
            nc.sync.dma_start(
                out=attn_d.ap()[b].rearrange("(q p) -> p q", p=P), in_=acoln_t[:]
            )

            cs_t = outp.tile([1, H], F32, name="cs_t")
            nc.vector.tensor_scalar_mul(cs_t[0:1, 0:512], ctx_ps0[:], rec_t[:])
            nc.vector.tensor_scalar_mul(cs_t[0:1, 512:1024], ctx_ps1[:], rec_t[:])
            nc.sync.dma_start(out=ctx_d.ap()[b : b + 1, :], in_=cs_t[:])

    nc.compile()
    return nc


def _prep_inputs(hidden, enc, W_w, W_b, U_w, U_b, V_w):
    bf16 = ml_dtypes.bfloat16
    uwT = np.ascontiguousarray(U_w.T).astype(bf16)
    wwT = np.ascontiguousarray(W_w.T).astype(np.float32)
    wub4 = np.ascontiguousarray(
        np.broadcast_to((W_b + U_b)[None, :], (BL, H))
    ).astype(np.float32)
    v128 = np.ascontiguousarray(
        np.broadcast_to(V_w.reshape(1, H), (P, H))
    ).astype(bf16)

    in_maps = []
    for i in range(NCORES):
        sl = slice(i * BL, (i + 1) * BL)
        e = enc[sl]
        in_maps.append(
            {
                "encT": np.ascontiguousarray(e.transpose(0, 2, 1)).astype(bf16),
                "encN": e.astype(bf16),
                "uwT": uwT,
                "wwT": wwT,
                "hidT": np.ascontiguousarray(hidden[sl, 0, :].T).astype(np.float32),
                "wub4": wub4,
                "v128": v128,
            }
        )
    return in_maps


def run(inputs: dict, trace: bool = False):
    """Build + run; returns ((context, attention), BassKernelResults)."""
    hidden = np.asarray(inputs["hidden"], dtype=np.float32)
    enc = np.asarray(inputs["encoder_output"], dtype=np.float32)
    W_w = np.asarray(inputs["W_w"], dtype=np.float32)
    W_b = np.asarray(inputs["W_b"], dtype=np.float32)
    U_w = np.asarray(inputs["U_w"], dtype=np.float32)
    U_b = np.asarray(inputs["U_b"], dtype=np.float32)
    V_w = np.asarray(inputs["V_w"], dtype=np.float32)
    V_b = np.asarray(inputs["V_b"], dtype=np.float32)

    nc = build_program(float(V_b.reshape(-1)[0]))
    in_maps = _prep_inputs(hidden, enc, W_w, W_b, U_w, U_b, V_w)
    res = run_bass_kernel_spmd(nc, in_maps, list(range(NCORES)), trace=trace)

    ctx = np.concatenate(
        [np.asarray(res.results[i]["ctx_out"]) for i in range(NCORES)], axis=0
    ).astype(np.float32)[:, None, :]
    attn = np.concatenate(
        [np.asarray(res.results[i]["attn_out"]) for i in range(NCORES)], axis=0
    ).astype(np.float32)[:, None, :]
    return (ctx, attn), res


def kernel(**inputs) -> tuple:
    out, _ = run(inputs, trace=False)
    return out


# revision 24
# speedup vs baseline: 1.4924x; 1.0168x over previous
"""Bahdanau attention on 8 Trainium2 NeuronCores.

Full inputs in, full outputs out. Batch (B=32) is sharded 4-per-core
(data parallel); all weights are replicated. Per core, for each of its
4 batches:

    e_proj[s, k]  = sum_h enc[b, s, h] * U_w[k, h]          (bf16 PE matmul,
                     s on partitions, k on the free axis)
    t[s, k]       = tanh(e_proj[s, k] + h_proj[b, k] + W_b[k] + U_b[k])
                     (DVE add of the broadcast row bias, then ACT tanh)
    scores[s]     = sum_k V[k] * t[s, k]                     (DVE fused
                     multiply + free-axis reduce — keeps the PE free)
    a[s]          = exp(scores[s] + V_b)                     (no max needed:
                     |scores| <= ||V||_1 + |V_b| ~ 26, exp fits fp32 easily)
    attention     = a / sum(a)        (sum via tiny ones-matmuls in PSUM)
    context[h]    = sum_s a[s] * enc[b, s, h] / sum(a)       (bf16 PE matmul)

The h-contraction needs encoder tiles with h on partitions while the
s-contraction needs s on partitions, so the host passes the encoder twice
(bf16 transposed + bf16 natural); 32 MB/core of DMA under ~280 us of PE work.
Scores emerge as 128-deep columns, which is exactly the lhsT layout the
context matmul needs — no on-chip transposes anywhere.
"""

from contextlib import ExitStack

import numpy as np
import ml_dtypes

import concourse.bass as bass  # noqa: F401
import concourse.mybir as mybir
from concourse import tile, bacc
from concourse.bass_utils import run_bass_kernel_spmd
from concourse.bass_isa import ReduceOp  # noqa: F401

BF16 = mybir.dt.bfloat16
F32 = mybir.dt.float32
AF = mybir.ActivationFunctionType
ALU = mybir.AluOpType

B, S, H = 32, 2048, 1024
NCORES = 8
BL = B // NCORES      # 4 batches per core
P = 128
KC = H // P           # 8 contraction chunks
SBLK = 4              # s-blocks per batch (DMA granularity)
SW = S // SBLK        # 512 rows per s-block
CH = SW // P          # 4 s-chunks of 128 per s-block
NQ = S // P           # 16 s-chunks per batch


def build_program(v_b: float) -> bacc.Bacc:
    nc = bacc.Bacc("TRN2", target_bir_lowering=False, debug=False, num_devices=NCORES)

    encT_d = nc.dram_tensor("encT", [BL, H, S], BF16, kind="ExternalInput")
    encN_d = nc.dram_tensor("encN", [BL, S, H], BF16, kind="ExternalInput")
    uwT_d = nc.dram_tensor("uwT", [H, H], BF16, kind="ExternalInput")
    wwT_d = nc.dram_tensor("wwT", [H, H], F32, kind="ExternalInput")
    hidT_d = nc.dram_tensor("hidT", [H, BL], F32, kind="ExternalInput")
    wub4_d = nc.dram_tensor("wub4", [BL, H], F32, kind="ExternalInput")
    v128_d = nc.dram_tensor("v128", [P, H], BF16, kind="ExternalInput")
    ctx_d = nc.dram_tensor("ctx_out", [BL, H], F32, kind="ExternalOutput")
    attn_d = nc.dram_tensor("attn_out", [BL, S], F32, kind="ExternalOutput")
    hrow_dram = nc.dram_tensor("hrow_scratch", [BL, H], F32)
    rec_dram = nc.dram_tensor("rec_scratch", [BL, 1], F32)

    with tile.TileContext(nc) as tc, ExitStack() as stack:
        const = stack.enter_context(tc.tile_pool(name="const", bufs=1))

        # sync (SP) HWDGE ring: the big bf16 operands the main loop needs first
        uw_t = const.tile([P, KC, H], BF16, name="uw_t")
        nc.sync.dma_start(
            out=uw_t[:], in_=uwT_d.ap().rearrange("(kc p) n -> p kc n", p=P)
        )
        # scalar (ACT) HWDGE ring: everything h_proj needs, in parallel
        hid_t = const.tile([P, KC, BL], F32, name="hid_t")
        nc.scalar.dma_start(
            out=hid_t[:], in_=hidT_d.ap().rearrange("(kc p) b -> p kc b", p=P)
        )
        wub4_t = const.tile([BL, H], F32, name="wub4_t")
        v128_t = const.tile([P, H], BF16, name="v128_t")
        one_t = const.tile([P, 1], BF16, name="one_t")
        nc.vector.memset(one_t[:], 1.0)
        vb_t = const.tile([P, 1], F32, name="vb_t")
        nc.vector.memset(vb_t[:], v_b)
        # hbrow[b, k] = h_proj[b, k] + W_b[k] + U_b[k]; hb128[.] broadcasts
        # the current batch's row across all 128 partitions (gpsimd).
        hbrow_t = const.tile([BL, H], F32, name="hbrow_t")

        # ---- phase 0: h_proj rows. lhsT = hidden columns (LDWEIGHTS is only
        # 4 cols) so this is 16 wide fp32 matmuls, nothing else.
        def emit_hproj():
            with (
                tc.tile_pool(name="wpool", bufs=1) as wpool,
                tc.tile_pool(name="hrow_psum", bufs=1, space="PSUM") as hrow_psum,
            ):
                ww_t = wpool.tile([P, KC, H], F32, name="ww_t")
                ww_src = wwT_d.ap().rearrange("(kc p) n -> p kc n", p=P)
                for kc in range(KC):
                    nc.scalar.dma_start(out=ww_t[:, kc, :], in_=ww_src[:, kc, :])
                nc.scalar.dma_start(out=wub4_t[:], in_=wub4_d.ap())
                nc.scalar.dma_start(out=v128_t[:], in_=v128_d.ap())
                hrow_ps = [
                    hrow_psum.tile([BL, 512], F32, name=f"hrow_ps{h}") for h in range(2)
                ]
                for kc in range(KC):
                    for h in range(2):
                        nc.tensor.matmul(
                            hrow_ps[h][:],
                            hid_t[:, kc, :],
                            ww_t[:, kc, h * 512 : (h + 1) * 512],
                            start=(kc == 0),
                            stop=(kc == KC - 1),
                        )
                for h in range(2):
                    nc.vector.tensor_add(
                        hbrow_t[:, h * 512 : (h + 1) * 512],
                        hrow_ps[h][:],
                        wub4_t[:, h * 512 : (h + 1) * 512],
                    )

        # ---- main pools (ep_psum first: disjoint banks from phase-0 pools)
        encTp = stack.enter_context(tc.tile_pool(name="encTp", bufs=2))
        encNp = stack.enter_context(tc.tile_pool(name="encNp", bufs=2))
        esp = stack.enter_context(tc.tile_pool(name="esp", bufs=3))
        tanhp = stack.enter_context(tc.tile_pool(name="tanhp", bufs=3))
        scrp = stack.enter_context(tc.tile_pool(name="scrp", bufs=2))
        accp = stack.enter_context(tc.tile_pool(name="accp", bufs=3))
        acolp = stack.enter_context(tc.tile_pool(name="acolp", bufs=2))
        hbp = stack.enter_context(tc.tile_pool(name="hbp", bufs=2))
        outp = stack.enter_context(tc.tile_pool(name="outp", bufs=2))
        ep_psum = stack.enter_context(tc.tile_pool(name="ep_psum", bufs=2, space="PSUM"))
        late = {}

        def late_pools():
            late["ctx"] = stack.enter_context(
                tc.tile_pool(name="ctx_psum", bufs=1, space="PSUM")
            )
            late["den"] = stack.enter_context(
                tc.tile_pool(name="den_psum", bufs=1, space="PSUM")
            )

        pending = []  # deferred PE work (ctx/den matmuls), staggered 2 chunks

        def drain_pending(keep=2):
            while len(pending) > keep:
                pending.pop(0)()

        emit_hproj()
        late_pools()

        for b in range(BL):
            # replicate this batch's bias row to all partitions via DRAM
            nc.sync.dma_start(out=hrow_dram.ap()[b : b + 1, :], in_=hbrow_t[b : b + 1, :])
            hb128_t = hbp.tile([P, H], F32, name="hb128_t")
            nc.sync.dma_start(
                out=hb128_t[:], in_=hrow_dram.ap()[b].partition_broadcast(P)
            )
            acol_t = acolp.tile([P, NQ], BF16, name="acol_t")
            cps = {}

            for sb in range(SBLK):
                encT_t = encTp.tile([P, KC, SW], BF16, name="encT_t")
                nc.sync.dma_start(
                    out=encT_t[:],
                    in_=encT_d.ap()[b].rearrange("(kc p) s -> p kc s", p=P)[
                        :, :, sb * SW : (sb + 1) * SW
                    ],
                )
                encN_t = encNp.tile([P, CH, H], BF16, name="encN_t")
                nc.sync.dma_start(
                    out=encN_t[:],
                    in_=encN_d.ap()[b].rearrange("(c p) h -> p c h", p=P)[
                        :, sb * CH : (sb + 1) * CH, :
                    ],
                )

                for c in range(CH):
                    q = sb * CH + c
                    # e_proj for 128 s-rows: [128, 1024] over two psum banks
                    ep_ps = ep_psum.tile([P, H], F32, name="ep_ps")
                    for kc in range(KC):
                        for h in range(2):
                            nc.tensor.matmul(
                                ep_ps[:, h * 512 : (h + 1) * 512],
                                encT_t[:, kc, c * P : (c + 1) * P],
                                uw_t[:, kc, h * 512 : (h + 1) * 512],
                                start=(kc == 0),
                                stop=(kc == KC - 1),
                            )

                    # + row bias, tanh, V-weighted free-axis reduce -> scores col
                    es_t = esp.tile([P, H], F32, name="es_t")
                    nc.vector.tensor_add(es_t[:], ep_ps[:], hb128_t[:])
                    tanh_t = tanhp.tile([P, H], BF16, name="tanh_t")
                    nc.scalar.activation(tanh_t[:], es_t[:], AF.Tanh)
                    scr_t = scrp.tile([P, H], BF16, name="scr_t")
                    nc.vector.tensor_mul(scr_t[:], tanh_t[:], v128_t[:])
                    acc_t = accp.tile([P, 1], F32, name="acc_t")
                    nc.vector.reduce_sum(acc_t[:], scr_t[:], axis=mybir.AxisListType.X)
                    # a-column = exp(scores + V_b), straight into lhsT layout
                    nc.scalar.activation(
                        acol_t[:, q : q + 1], acc_t[:], AF.Exp, bias=vb_t[:]
                    )

                    def emit_ctx(q=q, encN_t=encN_t, c=c):
                        if not cps:
                            cps["c0"] = late["ctx"].tile([1, 512], F32, name="ctx_ps0")
                            cps["c1"] = late["ctx"].tile([1, 512], F32, name="ctx_ps1")
                            cps["d"] = late["den"].tile([1, 1], F32, name="den_ps")
                        st = q == 0
                        sp = q == NQ - 1
                        nc.tensor.matmul(
                            cps["c0"][:], acol_t[:, q : q + 1], encN_t[:, c, 0:512],
                            start=st, stop=sp, skip_group_check=True,
                        )
                        nc.tensor.matmul(
                            cps["c1"][:], acol_t[:, q : q + 1], encN_t[:, c, 512:1024],
                            start=st, stop=sp, skip_group_check=True,
                        )
                        nc.tensor.matmul(
                            cps["d"][:], acol_t[:, q : q + 1], one_t[:],
                            start=st, stop=sp, skip_group_check=True,
                        )

                    pending.append(emit_ctx)
                    drain_pending(keep=2)

            drain_pending(keep=0)  # batch boundary: finish ctx/den for b
            ctx_ps0, ctx_ps1, den_ps = cps["c0"], cps["c1"], cps["d"]

            den_t = outp.tile([1, 1], F32, name="den_t")
            rec_t = outp.tile([1, 1], F32, name="rec_t")
            nc.vector.tensor_copy(den_t[:], den_ps[:])
            nc.vector.reciprocal(rec_t[:], den_t[:])
            rec128_t = accp.tile([P, 1], F32, name="rec128_t")
            nc.gpsimd.partition_broadcast(rec128_t[:], rec_t[:], channels=P)

            acoln_t = outp.tile([P, NQ], F32, name="acoln_t")
            nc.vector.tensor_scalar_mul(acoln_t[:], acol_t[:], rec128_t[:])
            nc.sync.dma_start(
                out=attn_d.ap()[b].rearrange("(q p) -> p q", p=P), in_=acoln_t[:]
            )

            cs_t = outp.tile([1, H], F32, name="cs_t")
            nc.vector.tensor_scalar_mul(cs_t[0:1, 0:512], ctx_ps0[:], rec_t[:])
            nc.vector.tensor_scalar_mul(cs_t[0:1, 512:1024], ctx_ps1[:], rec_t[:])
            nc.sync.dma_start(out=ctx_d.ap()[b : b + 1, :], in_=cs_t[:])

    nc.compile()
    return nc


def _prep_inputs(hidden, enc, W_w, W_b, U_w, U_b, V_w):
    bf16 = ml_dtypes.bfloat16
    uwT = np.ascontiguousarray(U_w.T).astype(bf16)
    wwT = np.ascontiguousarray(W_w.T).astype(np.float32)
    wub4 = np.ascontiguousarray(
        np.broadcast_to((W_b + U_b)[None, :], (BL, H))
    ).astype(np.float32)
    v128 = np.ascontiguousarray(
        np.broadcast_to(V_w.reshape(1, H), (P, H))
    ).astype(bf16)

    in_maps = []
    for i in range(NCORES):
        sl = slice(i * BL, (i + 1) * BL)
        e = enc[sl]
        in_maps.append(
            {
                "encT": np.ascontiguousarray(e.transpose(0, 2, 1)).astype(bf16),
                "encN": e.astype(bf16),
                "uwT": uwT,
                "wwT": wwT,
                "hidT": np.ascontiguousarray(hidden[sl, 0, :].T).astype(np.float32),
                "wub4": wub4,
                "v128": v128,
            }
        )
    return in_maps


def run(inputs: dict, trace: bool = False):
    """Build + run; returns ((context, attention), BassKernelResults)."""
    hidden = np.asarray(inputs["hidden"], dtype=np.float32)
    enc = np.asarray(inputs["encoder_output"], dtype=np.float32)
    W_w = np.asarray(inputs["W_w"], dtype=np.float32)
    W_b = np.asarray(inputs["W_b"], dtype=np.float32)
    U_w = np.asarray(inputs["U_w"], dtype=np.float32)
    U_b = np.asarray(inputs["U_b"], dtype=np.float32)
    V_w = np.asarray(inputs["V_w"], dtype=np.float32)
    V_b = np.asarray(inputs["V_b"], dtype=np.float32)

    nc = build_program(float(V_b.reshape(-1)[0]))
    in_maps = _prep_inputs(hidden, enc, W_w, W_b, U_w, U_b, V_w)
    res = run_bass_kernel_spmd(nc, in_maps, list(range(NCORES)), trace=trace)

    ctx = np.concatenate(
        [np.asarray(res.results[i]["ctx_out"]) for i in range(NCORES)], axis=0
    ).astype(np.float32)[:, None, :]
    attn = np.concatenate(
        [np.asarray(res.results[i]["attn_out"]) for i in range(NCORES)], axis=0
    ).astype(np.float32)[:, None, :]
    return (ctx, attn), res


def kernel(**inputs) -> tuple:
    out, _ = run(inputs, trace=False)
    return out


# revision 28
# speedup vs baseline: 1.5336x; 1.0276x over previous
"""Bahdanau attention on 8 Trainium2 NeuronCores.

Full inputs in, full outputs out. Batch (B=32) is sharded 4-per-core
(data parallel); all weights are replicated. Per core, for each of its
4 batches:

    e_proj[s, k]  = sum_h enc[b, s, h] * U_w[k, h]          (bf16 PE matmul,
                     s on partitions, k on the free axis)
    t[s, k]       = tanh(e_proj[s, k] + h_proj[b, k] + W_b[k] + U_b[k])
                     (DVE add of the broadcast row bias, then ACT tanh)
    scores[s]     = sum_k V[k] * t[s, k]                     (DVE fused
                     multiply + free-axis reduce — keeps the PE free)
    a[s]          = exp(scores[s] + V_b)                     (no max needed:
                     |scores| <= ||V||_1 + |V_b| ~ 26, exp fits fp32 easily)
    attention     = a / sum(a)        (sum via tiny ones-matmuls in PSUM)
    context[h]    = sum_s a[s] * enc[b, s, h] / sum(a)       (bf16 PE matmul)

The h-contraction needs encoder tiles with h on partitions while the
s-contraction needs s on partitions, so the host passes the encoder twice
(bf16 transposed + bf16 natural); 32 MB/core of DMA under ~280 us of PE work.
Scores emerge as 128-deep columns, which is exactly the lhsT layout the
context matmul needs — no on-chip transposes anywhere.
"""

from contextlib import ExitStack

import numpy as np
import ml_dtypes

import concourse.bass as bass  # noqa: F401
import concourse.mybir as mybir
from concourse import tile, bacc
from concourse.bass_utils import run_bass_kernel_spmd
from concourse.bass_isa import ReduceOp  # noqa: F401

BF16 = mybir.dt.bfloat16
F32 = mybir.dt.float32
AF = mybir.ActivationFunctionType
ALU = mybir.AluOpType

B, S, H = 32, 2048, 1024
NCORES = 8
BL = B // NCORES      # 4 batches per core
P = 128
KC = H // P           # 8 contraction chunks
SBLK = 4              # s-blocks per batch (DMA granularity)
SW = S // SBLK        # 512 rows per s-block
CH = SW // P          # 4 s-chunks of 128 per s-block
NQ = S // P           # 16 s-chunks per batch


def build_program(v_b: float) -> bacc.Bacc:
    nc = bacc.Bacc("TRN2", target_bir_lowering=False, debug=False, num_devices=NCORES)

    encT_d = nc.dram_tensor("encT", [BL, H, S], BF16, kind="ExternalInput")
    encN_d = nc.dram_tensor("encN", [BL, S, H], BF16, kind="ExternalInput")
    uwT_d = nc.dram_tensor("uwT", [H, H], BF16, kind="ExternalInput")
    wwT_d = nc.dram_tensor("wwT", [H, H], F32, kind="ExternalInput")
    hidT_d = nc.dram_tensor("hidT", [H, BL], F32, kind="ExternalInput")
    wub4_d = nc.dram_tensor("wub4", [BL, H], F32, kind="ExternalInput")
    v128_d = nc.dram_tensor("v128", [P, H], BF16, kind="ExternalInput")
    ctx_d = nc.dram_tensor("ctx_out", [BL, H], F32, kind="ExternalOutput")
    # column-major: attn_out[b, p, q] = attention[b, q*128 + p] (host reshapes)
    attn_d = nc.dram_tensor("attn_out", [BL, P, NQ], F32, kind="ExternalOutput")
    hrow_dram = nc.dram_tensor("hrow_scratch", [BL, H], F32)
    rec_dram = nc.dram_tensor("rec_scratch", [BL, 1], F32)

    with tile.TileContext(nc) as tc, ExitStack() as stack:
        const = stack.enter_context(tc.tile_pool(name="const", bufs=1))

        # sync (SP) HWDGE ring: the big bf16 operands the main loop needs first
        uw_t = const.tile([P, KC, H], BF16, name="uw_t")
        nc.sync.dma_start(
            out=uw_t[:], in_=uwT_d.ap().rearrange("(kc p) n -> p kc n", p=P)
        )
        # scalar (ACT) HWDGE ring: everything h_proj needs, in parallel
        hid_t = const.tile([P, KC, BL], F32, name="hid_t")
        nc.scalar.dma_start(
            out=hid_t[:], in_=hidT_d.ap().rearrange("(kc p) b -> p kc b", p=P)
        )
        wub4_t = const.tile([BL, H], F32, name="wub4_t")
        v128_t = const.tile([P, H], BF16, name="v128_t")
        one_t = const.tile([P, 1], BF16, name="one_t")
        nc.vector.memset(one_t[:], 1.0)
        vb_t = const.tile([P, 1], F32, name="vb_t")
        nc.vector.memset(vb_t[:], v_b)
        # hbrow[b, k] = h_proj[b, k] + W_b[k] + U_b[k]; hb128[.] broadcasts
        # the current batch's row across all 128 partitions (gpsimd).
        hbrow_t = const.tile([BL, H], F32, name="hbrow_t")

        # ---- phase 0: h_proj rows. lhsT = hidden columns (LDWEIGHTS is only
        # 4 cols) so this is 16 wide fp32 matmuls, nothing else.
        def emit_hproj():
            with (
                tc.tile_pool(name="wpool", bufs=1) as wpool,
                tc.tile_pool(name="hrow_psum", bufs=1, space="PSUM") as hrow_psum,
            ):
                ww_src = wwT_d.ap().rearrange("(kc p) n -> p kc n", p=P)
                ww_ts = []
                for kc in range(KC):
                    ww_kc = wpool.tile([P, H], F32, name=f"ww_{kc}")
                    nc.scalar.dma_start(out=ww_kc[:], in_=ww_src[:, kc, :])
                    ww_ts.append(ww_kc)
                nc.scalar.dma_start(out=wub4_t[:], in_=wub4_d.ap())
                nc.scalar.dma_start(out=v128_t[:], in_=v128_d.ap())
                hrow_ps = [
                    hrow_psum.tile([BL, 512], F32, name=f"hrow_ps{h}") for h in range(2)
                ]
                for kc in range(KC):
                    for h in range(2):
                        nc.tensor.matmul(
                            hrow_ps[h][:],
                            hid_t[:, kc, :],
                            ww_ts[kc][:, h * 512 : (h + 1) * 512],
                            start=(kc == 0),
                            stop=(kc == KC - 1),
                        )
                for h in range(2):
                    nc.vector.tensor_add(
                        hbrow_t[:, h * 512 : (h + 1) * 512],
                        hrow_ps[h][:],
                        wub4_t[:, h * 512 : (h + 1) * 512],
                    )

        # ---- main pools (ep_psum first: disjoint banks from phase-0 pools)
        encTp = stack.enter_context(tc.tile_pool(name="encTp", bufs=2))
        encNp = stack.enter_context(tc.tile_pool(name="encNp", bufs=2))
        esp = stack.enter_context(tc.tile_pool(name="esp", bufs=3))
        tanhp = stack.enter_context(tc.tile_pool(name="tanhp", bufs=3))
        scrp = stack.enter_context(tc.tile_pool(name="scrp", bufs=2))
        accp = stack.enter_context(tc.tile_pool(name="accp", bufs=3))
        acolp = stack.enter_context(tc.tile_pool(name="acolp", bufs=2))
        hbp = stack.enter_context(tc.tile_pool(name="hbp", bufs=2))
        outp = stack.enter_context(tc.tile_pool(name="outp", bufs=2))
        ep_psum = stack.enter_context(tc.tile_pool(name="ep_psum", bufs=2, space="PSUM"))
        late = {}

        def late_pools():
            late["ctx"] = stack.enter_context(
                tc.tile_pool(name="ctx_psum", bufs=1, space="PSUM")
            )
            late["den"] = stack.enter_context(
                tc.tile_pool(name="den_psum", bufs=1, space="PSUM")
            )

        pending = []  # deferred PE work (ctx/den matmuls), staggered 2 chunks

        def drain_pending(keep=2):
            while len(pending) > keep:
                pending.pop(0)()

        emit_hproj()
        late_pools()

        for b in range(BL):
            # replicate this batch's bias row to all partitions via DRAM
            nc.sync.dma_start(out=hrow_dram.ap()[b : b + 1, :], in_=hbrow_t[b : b + 1, :])
            hb128_t = hbp.tile([P, H], F32, name="hb128_t")
            nc.sync.dma_start(
                out=hb128_t[:], in_=hrow_dram.ap()[b].partition_broadcast(P)
            )
            acol_t = acolp.tile([P, NQ], BF16, name="acol_t")
            cps = {}

            for sb in range(SBLK):
                encT_t = encTp.tile([P, KC, SW], BF16, name="encT_t")
                nc.sync.dma_start(
                    out=encT_t[:],
                    in_=encT_d.ap()[b].rearrange("(kc p) s -> p kc s", p=P)[
                        :, :, sb * SW : (sb + 1) * SW
                    ],
                )
                encN_t = encNp.tile([P, CH, H], BF16, name="encN_t")
                nc.sync.dma_start(
                    out=encN_t[:],
                    in_=encN_d.ap()[b].rearrange("(c p) h -> p c h", p=P)[
                        :, sb * CH : (sb + 1) * CH, :
                    ],
                )

                for c in range(CH):
                    q = sb * CH + c
                    # e_proj for 128 s-rows: [128, 1024] over two psum banks
                    ep_ps = ep_psum.tile([P, H], F32, name="ep_ps")
                    for kc in range(KC):
                        for h in range(2):
                            nc.tensor.matmul(
                                ep_ps[:, h * 512 : (h + 1) * 512],
                                encT_t[:, kc, c * P : (c + 1) * P],
                                uw_t[:, kc, h * 512 : (h + 1) * 512],
                                start=(kc == 0),
                                stop=(kc == KC - 1),
                            )

                    # + row bias, tanh, V-weighted free-axis reduce -> scores col
                    es_t = esp.tile([P, H], F32, name="es_t")
                    nc.vector.tensor_add(es_t[:], ep_ps[:], hb128_t[:])
                    tanh_t = tanhp.tile([P, H], BF16, name="tanh_t")
                    nc.scalar.activation(tanh_t[:], es_t[:], AF.Tanh)
                    scr_t = scrp.tile([P, H], BF16, name="scr_t")
                    nc.vector.tensor_mul(scr_t[:], tanh_t[:], v128_t[:])
                    acc_t = accp.tile([P, 1], F32, name="acc_t")
                    nc.vector.reduce_sum(acc_t[:], scr_t[:], axis=mybir.AxisListType.X)
                    # a-column = exp(scores + V_b), straight into lhsT layout
                    nc.scalar.activation(
                        acol_t[:, q : q + 1], acc_t[:], AF.Exp, bias=vb_t[:]
                    )

                    def emit_ctx(q=q, encN_t=encN_t, c=c):
                        if not cps:
                            cps["c0"] = late["ctx"].tile([1, 512], F32, name="ctx_ps0")
                            cps["c1"] = late["ctx"].tile([1, 512], F32, name="ctx_ps1")
                            cps["d"] = late["den"].tile([1, 1], F32, name="den_ps")
                        st = q == 0
                        sp = q == NQ - 1
                        nc.tensor.matmul(
                            cps["c0"][:], acol_t[:, q : q + 1], encN_t[:, c, 0:512],
                            start=st, stop=sp, skip_group_check=True,
                        )
                        nc.tensor.matmul(
                            cps["c1"][:], acol_t[:, q : q + 1], encN_t[:, c, 512:1024],
                            start=st, stop=sp, skip_group_check=True,
                        )
                        nc.tensor.matmul(
                            cps["d"][:], acol_t[:, q : q + 1], one_t[:],
                            start=st, stop=sp, skip_group_check=True,
                        )

                    pending.append(emit_ctx)
                    drain_pending(keep=2)

            drain_pending(keep=0)  # batch boundary: finish ctx/den for b
            ctx_ps0, ctx_ps1, den_ps = cps["c0"], cps["c1"], cps["d"]

            den_t = outp.tile([1, 1], F32, name="den_t")
            rec_t = outp.tile([1, 1], F32, name="rec_t")
            nc.vector.tensor_copy(den_t[:], den_ps[:])
            nc.vector.reciprocal(rec_t[:], den_t[:])
            rec128_t = accp.tile([P, 1], F32, name="rec128_t")
            nc.gpsimd.partition_broadcast(rec128_t[:], rec_t[:], channels=P)

            acoln_t = outp.tile([P, NQ], F32, name="acoln_t")
            nc.vector.tensor_scalar_mul(acoln_t[:], acol_t[:], rec128_t[:])
            nc.sync.dma_start(out=attn_d.ap()[b], in_=acoln_t[:])

            cs_t = outp.tile([1, H], F32, name="cs_t")
            nc.vector.tensor_scalar_mul(cs_t[0:1, 0:512], ctx_ps0[:], rec_t[:])
            nc.vector.tensor_scalar_mul(cs_t[0:1, 512:1024], ctx_ps1[:], rec_t[:])
            nc.sync.dma_start(out=ctx_d.ap()[b : b + 1, :], in_=cs_t[:])

    nc.compile()
    return nc


def _prep_inputs(hidden, enc, W_w, W_b, U_w, U_b, V_w):
    bf16 = ml_dtypes.bfloat16
    uwT = np.ascontiguousarray(U_w.T).astype(bf16)
    wwT = np.ascontiguousarray(W_w.T).astype(np.float32)
    wub4 = np.ascontiguousarray(
        np.broadcast_to((W_b + U_b)[None, :], (BL, H))
    ).astype(np.float32)
    v128 = np.ascontiguousarray(
        np.broadcast_to(V_w.reshape(1, H), (P, H))
    ).astype(bf16)

    in_maps = []
    for i in range(NCORES):
        sl = slice(i * BL, (i + 1) * BL)
        e = enc[sl]
        in_maps.append(
            {
                "encT": np.ascontiguousarray(e.transpose(0, 2, 1)).astype(bf16),
                "encN": e.astype(bf16),
                "uwT": uwT,
                "wwT": wwT,
                "hidT": np.ascontiguousarray(hidden[sl, 0, :].T).astype(np.float32),
                "wub4": wub4,
                "v128": v128,
            }
        )
    return in_maps


def run(inputs: dict, trace: bool = False):
    """Build + run; returns ((context, attention), BassKernelResults)."""
    hidden = np.asarray(inputs["hidden"], dtype=np.float32)
    enc = np.asarray(inputs["encoder_output"], dtype=np.float32)
    W_w = np.asarray(inputs["W_w"], dtype=np.float32)
    W_b = np.asarray(inputs["W_b"], dtype=np.float32)
    U_w = np.asarray(inputs["U_w"], dtype=np.float32)
    U_b = np.asarray(inputs["U_b"], dtype=np.float32)
    V_w = np.asarray(inputs["V_w"], dtype=np.float32)
    V_b = np.asarray(inputs["V_b"], dtype=np.float32)

    nc = build_program(float(V_b.reshape(-1)[0]))
    in_maps = _prep_inputs(hidden, enc, W_w, W_b, U_w, U_b, V_w)
    res = run_bass_kernel_spmd(nc, in_maps, list(range(NCORES)), trace=trace)

    ctx = np.concatenate(
        [np.asarray(res.results[i]["ctx_out"]) for i in range(NCORES)], axis=0
    ).astype(np.float32)[:, None, :]
    attn_col = np.concatenate(
        [np.asarray(res.results[i]["attn_out"]) for i in range(NCORES)], axis=0
    ).astype(np.float32)
    # attn_col[b, p, q] holds attention[b, q*128 + p]
    attn = attn_col.transpose(0, 2, 1).reshape(B, S)[:, None, :]
    return (ctx, attn), res


def kernel(**inputs) -> tuple:
    out, _ = run(inputs, trace=False)
    return out
